# revision 4
# baseline (speedup 1.0000x reference)
"""AUGRU (DIEN attention layer) on 8 Trainium2 NeuronCores via Bass/Tile.

Problem: B=2048, T=200, D=128, H=128 fp32 AUGRU scan with per-row sequence
lengths (zero output + state carry past seq_len).

Strategy (pure batch data parallelism, 256 rows per core):
  - All on-chip tiles use the "transposed" layout [feature=128 partitions,
    batch on the free dim], so the recurrent matmuls keep the weight matrices
    as the stationary operand (lhsT) and h as the moving operand, and the
    device never transposes anything.
  - The host pre-transposes x to [D, T, B_local], pre-multiplies attention
    scores by the validity mask (t < seq_len) and replicates them across the
    128 partitions, so the masked state carry falls out of the recurrence
    (u' = 0 -> h' = h) and the per-step device work is 6 matmuls, 2
    activations and 6 elementwise ops.
  - Matmuls run in float32r (full PE rate at moving dim 256, ~tf32-pair
    precision); everything accumulates in fp32 PSUM. End-to-end absmax error
    vs the fp32 reference is ~5e-4 relative to the output scale.
  - Output is written as [T, H, B_local] (contiguous per-partition lines) and
    transposed back on the host during unsharding.
"""

import os

import numpy as np
import ml_dtypes

import concourse.bacc as bacc
import concourse.mybir as mybir
import concourse.tile as tile
from concourse.bass_utils import run_bass_kernel_spmd

F32 = mybir.dt.float32
BF16 = mybir.dt.bfloat16
F32R = mybir.dt.float32r
AF = mybir.ActivationFunctionType
OP = mybir.AluOpType

B, T, D, H = 2048, 200, 128, 128
NCORES = 8
BL = B // NCORES  # 256 batch rows per core
TB = 25           # timesteps per DMA block

LAST_EXEC_TIME_NS = None
_NC_CACHE = {}


def _build_kernel_f32(bg_const, bc_const):
    nc = bacc.Bacc("TRN2", target_bir_lowering=False, debug=False, num_devices=NCORES)

    xT = nc.dram_tensor("xT", [128, T * BL], F32R, kind="ExternalInput")
    am = nc.dram_tensor("am", [128, T * BL], F32, kind="ExternalInput")
    mk = nc.dram_tensor("mk", [128, T * BL], BF16, kind="ExternalInput")
    wnames = ["wxr", "whr", "wxu", "whu", "wxc", "whc"]
    wd = {n: nc.dram_tensor(n, [128, 128], F32R, kind="ExternalInput") for n in wnames}
    bgr = nc.dram_tensor("bgr", [128, 1], F32, kind="ExternalInput")
    bgu = nc.dram_tensor("bgu", [128, 1], F32, kind="ExternalInput")
    bcv = nc.dram_tensor("bcv", [128, 1], F32, kind="ExternalInput")
    h0d = nc.dram_tensor("h0", [128, BL], F32R, kind="ExternalInput")
    outT = nc.dram_tensor("outT", [T, 128, BL], F32, kind="ExternalOutput")

    with tile.TileContext(nc) as tc:
        with (
            tc.tile_pool(name="w", bufs=1) as wpool,
            tc.tile_pool(name="xb", bufs=2) as xpool,
            tc.tile_pool(name="ab", bufs=2) as apool,
            tc.tile_pool(name="mb", bufs=2) as mpool,
            tc.tile_pool(name="h", bufs=2) as hpool,
            tc.tile_pool(name="s", bufs=3) as spool,
            tc.tile_pool(name="o", bufs=6) as opool,
            tc.tile_pool(name="ps", bufs=2, space="PSUM") as ppool,
        ):
            w = {}
            for n in wnames:
                wt = wpool.tile([128, 128], F32R, tag=n, name=f"w_{n}")
                nc.sync.dma_start(wt[:], wd[n].ap())
                w[n] = wt
            btiles = {}
            if bg_const is None:
                for n, dt_ in (("bgr", bgr), ("bgu", bgu)):
                    bt = wpool.tile([128, 1], F32, tag=n, name=f"b_{n}")
                    nc.sync.dma_start(bt[:], dt_.ap())
                    btiles[n] = bt
            if bc_const is None:
                bt = wpool.tile([128, 1], F32, tag="bcv", name="b_bcv")
                nc.sync.dma_start(bt[:], bcv.ap())
                btiles["bcv"] = bt
            bias_r = bg_const if bg_const is not None else btiles["bgr"][:]
            bias_u = bg_const if bg_const is not None else btiles["bgu"][:]
            bias_c = bc_const if bc_const is not None else btiles["bcv"][:]

            h = hpool.tile([128, BL], F32R, tag="h")
            nc.sync.dma_start(h[:], h0d.ap())

            mm = nc.tensor.matmul
            tt = nc.vector.tensor_tensor
            gt = nc.gpsimd.tensor_tensor
            for blk in range(T // TB):
                lo, hi = blk * TB * BL, (blk + 1) * TB * BL
                xb = xpool.tile([128, TB * BL], F32R, tag="xb")
                nc.sync.dma_start(xb[:], xT.ap()[:, lo:hi])
                ab = apool.tile([128, TB * BL], F32, tag="ab")
                nc.sync.dma_start(ab[:], am.ap()[:, lo:hi])
                mb = mpool.tile([128, TB * BL], BF16, tag="mb")
                nc.sync.dma_start(mb[:], mk.ap()[:, lo:hi])
                for tl in range(TB):
                    t = blk * TB + tl
                    off = tl * BL
                    xt = xb[:, off:off + BL]
                    # r, u and the candidate each get their own PSUM bank /
                    # accumulation group, so sigmoid(r) unblocks as soon as the
                    # r-side matmuls finish.
                    pr = ppool.tile([128, 256], F32, tag="pr", name=f"pr_{t}")
                    pu = ppool.tile([128, 256], F32, tag="pu", name=f"pu_{t}")
                    pc = ppool.tile([128, 256], F32, tag="pc", name=f"pc_{t}")
                    mm(pr[:], w["wxr"][:], xt, start=True, stop=False)
                    mm(pr[:], w["whr"][:], h[:], start=False, stop=True)
                    mm(pu[:], w["wxu"][:], xt, start=True, stop=False)
                    mm(pu[:], w["whu"][:], h[:], start=False, stop=True)
                    mm(pc[:], w["wxc"][:], xt, start=True, stop=False)
                    # ah = am*h at cycle start (GPSIMD, off the recurrence)
                    ah = spool.tile([128, BL], F32, tag="ah", name=f"ah_{t}")
                    gt(ah[:], ab[:, off:off + BL], h[:], OP.mult)
                    r32 = spool.tile([128, BL], F32, tag="r32", name=f"r_{t}")
                    nc.scalar.activation(r32[:], pr[:], AF.Sigmoid, bias=bias_r)
                    rh = spool.tile([128, BL], F32R, tag="rh", name=f"rh_{t}")
                    tt(rh[:], r32[:], h[:], OP.mult)
                    mm(pc[:], w["whc"][:], rh[:], start=False, stop=True)
                    u32 = spool.tile([128, BL], F32, tag="u32", name=f"u_{t}")
                    nc.scalar.activation(u32[:], pu[:], AF.Sigmoid, bias=bias_u)
                    # u-branch: q = u*(am*h) and wv = h - q finish before tanh;
                    # up = u*am (for v = u'*c) runs on GPSIMD in parallel, so
                    # after tanh only two DVE ops remain on the recurrence.
                    q = spool.tile([128, BL], F32, tag="q", name=f"q_{t}")
                    tt(q[:], u32[:], ah[:], OP.mult)
                    up = spool.tile([128, BL], F32, tag="up", name=f"up_{t}")
                    gt(up[:], u32[:], ab[:, off:off + BL], OP.mult)
                    wv = spool.tile([128, BL], F32, tag="wv", name=f"wv_{t}")
                    tt(wv[:], h[:], q[:], OP.subtract)
                    cc = spool.tile([128, BL], F32, tag="cc", name=f"cc_{t}")
                    nc.scalar.activation(cc[:], pc[:], AF.Tanh, bias=bias_c)
                    v = spool.tile([128, BL], F32, tag="v", name=f"v_{t}")
                    tt(v[:], up[:], cc[:], OP.mult)
                    hn = hpool.tile([128, BL], F32R, tag="h", name=f"h_{t}")
                    tt(hn[:], wv[:], v[:], OP.add)
                    h = hn
                    ot = opool.tile([128, BL], F32, tag="o", name=f"o_{t}")
                    gt(ot[:], mb[:, off:off + BL], hn[:], OP.mult)
                    nc.sync.dma_start(outT.ap()[t, :, :], ot[:])
    nc.compile()
    return nc


def _prep_inputs(inputs, att_scores, seq_len, Wg, bg, Wc, bc):
    x = np.ascontiguousarray(np.asarray(inputs, dtype=np.float32))
    att = np.asarray(att_scores, dtype=np.float32)
    sl = np.asarray(seq_len, dtype=np.int32)
    Wg = np.asarray(Wg, dtype=np.float32)
    bg = np.asarray(bg, dtype=np.float32)
    Wc = np.asarray(Wc, dtype=np.float32)
    bc = np.asarray(bc, dtype=np.float32)

    m = (np.arange(T, dtype=np.int32)[None, :] < sl[:, None]).astype(np.float32)
    am = att * m
    m16 = m.astype(ml_dtypes.bfloat16)

    bg_const = float(bg.flat[0]) if np.all(bg == bg.flat[0]) else None
    bc_const = float(bc.flat[0]) if np.all(bc == bc.flat[0]) else None

    wmats = {
        "wxr": Wg[0:128, 0:128], "whr": Wg[128:256, 0:128],
        "wxu": Wg[0:128, 128:256], "whu": Wg[128:256, 128:256],
        "wxc": Wc[0:128, :], "whc": Wc[128:256, :],
    }
    wmats = {k: np.ascontiguousarray(v, dtype=np.float32) for k, v in wmats.items()}
    bgr = np.ascontiguousarray(bg[0:128, None])
    bgu = np.ascontiguousarray(bg[128:256, None])
    bcv = np.ascontiguousarray(bc[:, None])
    h0 = np.zeros((128, BL), np.float32)

    in_maps = []
    for k in range(NCORES):
        s = slice(k * BL, (k + 1) * BL)
        xk = np.ascontiguousarray(x[s].transpose(2, 1, 0))          # [D, T, BL]
        amk = np.ascontiguousarray(
            np.broadcast_to(am[s].T[None, :, :], (128, T, BL)))     # [128, T, BL]
        mkk = np.ascontiguousarray(
            np.broadcast_to(m16[s].T[None, :, :], (128, T, BL)))
        in_maps.append({
            "xT": xk.reshape(128, T * BL),
            "am": amk.reshape(128, T * BL),
            "mk": mkk.reshape(128, T * BL),
            **wmats,
            "bgr": bgr, "bgu": bgu, "bcv": bcv, "h0": h0,
        })
    return in_maps, bg_const, bc_const


def kernel(inputs, att_scores, seq_len, Wg, bg, Wc, bc):
    global LAST_EXEC_TIME_NS
    in_maps, bg_const, bc_const = _prep_inputs(
        inputs, att_scores, seq_len, Wg, bg, Wc, bc)

    key = (bg_const, bc_const)
    if key not in _NC_CACHE:
        _NC_CACHE[key] = _build_kernel_f32(bg_const, bc_const)
    nc = _NC_CACHE[key]

    trace = bool(int(os.environ.get("AUGRU_TRACE", "0")))
    kwargs = {}
    if trace:
        kwargs["trace"] = True
        tmpdir = os.environ.get("AUGRU_TRACE_DIR")
        if tmpdir:
            os.makedirs(tmpdir, exist_ok=True)
            kwargs["tmpdir"] = tmpdir
    try:
        res = run_bass_kernel_spmd(nc, in_maps, list(range(NCORES)), **kwargs)
    except Exception:
        if not kwargs:
            raise
        # profiling is best-effort; retry without it
        res = run_bass_kernel_spmd(nc, in_maps, list(range(NCORES)))
    LAST_EXEC_TIME_NS = res.exec_time_ns

    out = np.empty((B, T, H), np.float32)
    for k in range(NCORES):
        o = res.results[k]["outT"]                       # [T, H, BL]
        out[k * BL:(k + 1) * BL] = o.transpose(2, 0, 1)  # [BL, T, H]
    return out
